# revision 1
# baseline (speedup 1.0000x reference)
"""Multi-head attention forward on 8 Trainium2 NeuronCores (Bass/Tile).

Problem: B=2, S=2048, d_model=1024, 16 heads (depth 64), fp32.
  q/k/v = query @ W{q,k,v}; logits = q k^T / 8 + mask * -1e9;
  out = softmax(logits) v @ Wo.

Sharding (Megatron-style, hardcoded): core c handles batch b = c//4 and head
group hg = c%4 (4 heads = 256 of the 1024 head dims). Wq/Wk/Wv are
column-sharded, Wo row-sharded; each core emits a partial [S, 1024] output and
the host sums the 4 partials per batch (the "all-reduce").

Per-core kernel design:
  * All attention math runs transposed: qT/kT are [depth, S] (f32r) so QK^T
    lands as logitsT [k, q] tiles straight off the PE with no transposes,
    and AV^T = V.T(lhsT) @ expT needs none either.
  * ScalarE turns logits psum directly into exp weights (scale folded in);
    VectorE multiplies by (1-mask) in bf16 (2x mode). exp weights and V are
    bf16; QK^T and the output projection stay f32r for accuracy.
  * The softmax denominator comes free from a ones-column appended to V
    (row 64 of the AV psum accumulator); reciprocals are computed
    partition-major via tiny PE transposes and broadcast back with a
    rank-1 f32r matmul.
  * The mask streams in 16 split DMAs so the first tiles land early; the
    output projection for each 1024-wide q-chunk is folded into the main
    loop so it overlaps the next chunk's attention; a short identity-matmul
    spin warms the PE clock (HAM) while the first DMAs land.
"""

import sys

import numpy as np

sys.path.insert(0, "/opt/trn_rl_repo")

B = 2
S = 2048
D = 1024
HEADS = 16
DEPTH = 64
CORES = 8
HG = 4          # head groups (cores per batch)
HPC = 4         # heads per core
DH = HPC * DEPTH  # per-core head width = 256

_CACHE = {}


def _build_program():
    import concourse.bass as bass  # noqa: F401  (registers engines)
    import concourse.mybir as mybir
    import concourse.tile as tile
    from concourse import bacc
    from concourse.bass_interp import get_hw_module
    from concourse.masks import make_identity

    dt = mybir.dt
    f32, f32r, bf16 = dt.float32, dt.float32r, dt.bfloat16
    MULT = mybir.AluOpType.mult
    EXP = mybir.ActivationFunctionType.Exp

    nc = bacc.Bacc(
        "TRN2",
        target_bir_lowering=False,
        debug=False,
        enable_asserts=True,
        num_devices=CORES,
    )

    xT = nc.dram_tensor("xT", [D, S], f32r, kind="ExternalInput").ap()
    imaskT = nc.dram_tensor("imaskT", [S, S], bf16, kind="ExternalInput").ap()
    wq = nc.dram_tensor("wq", [D, DH], f32r, kind="ExternalInput").ap()
    wk = nc.dram_tensor("wk", [D, DH], f32r, kind="ExternalInput").ap()
    wv = nc.dram_tensor("wv", [D, DH], f32r, kind="ExternalInput").ap()
    wo = nc.dram_tensor("wo", [DH, D], f32r, kind="ExternalInput").ap()
    vones = nc.dram_tensor("vones", [128, HPC, 1], bf16, kind="ExternalInput").ap()
    ones_rd = nc.dram_tensor("ones_rd", [1, DEPTH], f32r, kind="ExternalInput").ap()
    out = nc.dram_tensor("out", [S, D], f32, kind="ExternalOutput").ap()

    
    with tile.TileContext(nc) as tc:
        with tc.tile_pool(name="persist", bufs=1) as pp:
            # Persistent SBUF tiles.
            qT = [pp.tile([128, S], f32r, tag=f"qT{g}", name=f"qT{g}") for g in range(2)]
            kT = [pp.tile([128, S], f32r, tag=f"kT{g}", name=f"kT{g}") for g in range(2)]
            vt = [pp.tile([128, HPC, DEPTH + 1], bf16, tag=f"v{i}", name=f"v{i}") for i in range(16)]
            wot = [pp.tile([128, D], f32r, tag=f"wo{g}", name=f"wo{g}") for g in range(2)]
            ident = pp.tile([128, 128], f32, tag="ident", name="ident")
            ones_r = pp.tile([1, DEPTH], f32r, tag="ones_r", name="ones_r")
            one1 = pp.tile([1, 1], f32, tag="one1", name="one1")

            make_identity(nc, ident[:])
            with tc.tile_pool(name="psW", bufs=2, space="PSUM") as psW:
                for w in range(80):
                    psw = psW.tile([128, 128], f32, tag="warm", name="warm")
                    nc.tensor.matmul(psw[:], ident[:], ident[:],
                                     start=True, stop=True)
            nc.sync.dma_start(ones_r[:], ones_rd[:])
            nc.gpsimd.memset(one1[:], 1.0)
            for g in range(2):
                nc.sync.dma_start(wot[g][:], wo[g * 128:(g + 1) * 128, :])

            # ---- Phase 1: projections (xT is query[b].T, fed transposed from host)
            with tc.tile_pool(name="xw", bufs=1) as xw, \
                 tc.tile_pool(name="psA", bufs=4, space="PSUM") as psA:
                xt = [xw.tile([128, S], f32r, tag=f"x{d}", name=f"x{d}") for d in range(8)]
                wts = {}
                for nm, srcd in (("wq", wq), ("wk", wk), ("wv", wv)):
                    wts[nm] = [xw.tile([128, DH], f32r, tag=f"{nm}{d}", name=f"{nm}{d}") for d in range(8)]
                for d in range(8):
                    nc.sync.dma_start(wts["wq"][d][:], wq[d * 128:(d + 1) * 128, :])
                for d in range(8):
                    nc.sync.dma_start(xt[d][:], xT[d * 128:(d + 1) * 128, :])
                for nm, srcd in (("wk", wk), ("wv", wv)):
                    for d in range(8):
                        nc.sync.dma_start(wts[nm][d][:], srcd[d * 128:(d + 1) * 128, :])

                # qT/kT: [dh, s] = Wq^T-slice . xT, accumulated over 8 D-chunks.
                for wt, dst in ((wts["wq"], qT), (wts["wk"], kT)):
                    for g in range(2):
                        for sc in range(4):
                            ps = psA.tile([128, 512], f32, tag="proj", name="proj")
                            for d in range(8):
                                nc.tensor.matmul(
                                    ps[:],
                                    wt[d][:, g * 128:(g + 1) * 128],
                                    xt[d][:, sc * 512:(sc + 1) * 512],
                                    start=(d == 0), stop=(d == 7),
                                )
                            nc.vector.tensor_copy(dst[g][:, sc * 512:(sc + 1) * 512], ps[:])

                # v: natural [s, dh] layout, stored per 128-row tile as
                # [128, head, 65] with a ones column at index 64 (denominator).
                for st in range(16):
                    ps = psA.tile([128, DH], f32, tag="proj", name="proj")
                    for d in range(8):
                        nc.tensor.matmul(
                            ps[:],
                            xt[d][:, st * 128:(st + 1) * 128],
                            wts["wv"][d][:],
                            start=(d == 0), stop=(d == 7),
                        )
                    nc.sync.dma_start(
                        vt[st][:, :, DEPTH:DEPTH + 1],
                        vones[:],
                    )
                    nc.vector.tensor_copy(
                        vt[st][:, :, 0:DEPTH],
                        ps[:].rearrange("p (h e) -> p h e", h=HPC),
                    )

            # ---- Phase 2: attention, fully transposed ----
            # Inner loop touches only PE (logits, mask-add via -1e9-identity
            # accumulate, AV^T) and ScalarE (exp psum->sbuf). VectorE only
            # handles the per-head epilogue (attnT copies, reciprocal, norm).
            attnT = [pp.tile([128, S], f32r, tag=f"attnT{g}", name=f"attnT{g}") for g in range(2)]
            with tc.tile_pool(name="attn", bufs=2) as ab, \
                 tc.tile_pool(name="exs", bufs=3) as exs, \
                 tc.tile_pool(name="psL", bufs=2, space="PSUM") as psL, \
                 tc.tile_pool(name="psO", bufs=1, space="PSUM") as psO, \
                 tc.tile_pool(name="psB", bufs=1, space="PSUM") as psB:
                mt = ab.tile([128, 16, S], bf16, tag="mask", name="mask", bufs=1)
                imaskT_r = imaskT.rearrange("(t p) q -> p t q", p=128)
                for kb in range(16):
                    nc.sync.dma_start(mt[:, kb:kb + 1, :], imaskT_r[:, kb:kb + 1, :])
                for qcp in range(2):
                    qs = slice(qcp * 1024, (qcp + 1) * 1024)
                    dden = ab.tile([1, HPC, 1024], f32, tag="dden", name="dden", bufs=1)
                    for h in range(HPC):
                        g, po = h // 2, (h % 2) * 64
                        pso = psO.tile([65, 1024], f32, tag="av", name="av")
                        for kb in range(16):
                            psl = psL.tile([128, 1024], f32, tag="lg", name="lg")
                            for half in range(2):
                                hs = slice(half * 512, (half + 1) * 512)
                                qh = slice(qcp * 1024 + half * 512,
                                           qcp * 1024 + half * 512 + 512)
                                nc.tensor.matmul(
                                    psl[:, hs],
                                    kT[g][po:po + 64, kb * 128:(kb + 1) * 128],
                                    qT[g][po:po + 64, qh],
                                    start=True, stop=True,
                                )
                            ex = exs.tile([128, 1024], bf16, tag="ex", name="ex", bufs=4)
                            nc.scalar.activation(ex[:], psl[:], EXP, scale=0.125)
                            em = exs.tile([128, 1024], bf16, tag="em", name="em", bufs=8)
                            nc.vector.tensor_tensor(em[:], ex[:], mt[:, kb, qs], MULT)
                            for half in range(2):
                                hs = slice(half * 512, (half + 1) * 512)
                                nc.tensor.matmul(
                                    pso[:, hs], vt[kb][:, h, :], em[:, hs],
                                    start=(kb == 0), stop=(kb == 15),
                                )
                        nc.vector.tensor_copy(attnT[g][po:po + 64, qs], pso[0:64, :])
                        nc.vector.tensor_copy(dden[0:1, h, :], pso[64:65, :])

                    # Reciprocal of the 4x1024 denominators: transpose the
                    # single-partition rows into partition-major [128, 32]
                    # columns with tiny PE transposes, reciprocal once, and
                    # transpose back.
                    dflat = dden.rearrange("p h q -> p (h q)")
                    pst = psB.tile([128, 32], f32, tag="dt", name="dt", bufs=2)
                    for c in range(32):
                        nc.tensor.transpose(
                            pst[:, c:c + 1],
                            dflat[0:1, c * 128:(c + 1) * 128],
                            one1[:],
                        )
                    rT = ab.tile([128, 32], f32, tag="rT", name="rT")
                    nc.vector.reciprocal(rT[:], pst[:])
                    rden = [ab.tile([1, 1024], f32r, tag=f"rden{h}", name=f"rden{h}", bufs=1) for h in range(HPC)]
                    for h in range(HPC):
                        g, po = h // 2, (h % 2) * 64
                        for half in range(2):
                            hs = slice(half * 512, (half + 1) * 512)
                            qh = slice(qcp * 1024 + half * 512,
                                       qcp * 1024 + half * 512 + 512)
                            psb = psB.tile([1, 512], f32, tag="dt", name="rdt", bufs=2)
                            for qb in range(4):
                                c = h * 8 + half * 4 + qb
                                nc.tensor.transpose(
                                    psb[0:1, qb * 128:(qb + 1) * 128],
                                    rT[:, c:c + 1],
                                    ident[:],
                                )
                            nc.vector.tensor_copy(rden[h][0:1, hs], psb[:])
                            psc = psB.tile([64, 512], f32, tag="dt", name="psc", bufs=2)
                            nc.tensor.matmul(
                                psc[:], ones_r[:], rden[h][0:1, hs],
                                start=True, stop=True,
                            )
                            nc.vector.tensor_tensor(
                                attnT[g][po:po + 64, qh],
                                attnT[g][po:po + 64, qh], psc[:], MULT,
                            )

                    # Output projection for this qcp's s-range (overlaps
                    # with the next qcp's attention on the other engines).
                    for st in range(qcp * 8, qcp * 8 + 8):
                        ot = ab.tile([128, D], f32, tag="ot", name="ot", bufs=2)
                        for nch in range(2):
                            psf = psB.tile([128, 512], f32, tag="dt", name="po", bufs=2)
                            for g in range(2):
                                nc.tensor.matmul(
                                    psf[:],
                                    attnT[g][:, st * 128:(st + 1) * 128],
                                    wot[g][:, nch * 512:(nch + 1) * 512],
                                    start=(g == 0), stop=(g == 1),
                                )
                            nc.vector.tensor_copy(ot[:, nch * 512:(nch + 1) * 512], psf[:])
                        nc.sync.dma_start(out[st * 128:(st + 1) * 128, :], ot[:])

    nc.compile()
    nc.m = get_hw_module(nc.m)
    return nc


def _get_program():
    if "nc" not in _CACHE:
        _CACHE["nc"] = _build_program()
    return _CACHE["nc"]


def _make_in_maps(query, attention_mask, Wq, Wk, Wv, Wo):
    import ml_dtypes

    in_maps = []
    imaskT_b = []
    xT_b = []
    for b in range(B):
        imaskT_b.append(
            np.ascontiguousarray(1 - attention_mask[b, 0].T).astype(ml_dtypes.bfloat16)
        )
        xT_b.append(np.ascontiguousarray(query[b].T))
    for c in range(CORES):
        b, hg = c // HG, c % HG
        cs = slice(hg * DH, (hg + 1) * DH)
        in_maps.append({
            "xT": xT_b[b],
            "imaskT": imaskT_b[b],
            "wq": np.ascontiguousarray(Wq[:, cs]),
            "wk": np.ascontiguousarray(Wk[:, cs]),
            "wv": np.ascontiguousarray(Wv[:, cs]),
            "wo": np.ascontiguousarray(Wo[cs, :]),
            "vones": np.ones((128, HPC, 1), dtype=ml_dtypes.bfloat16),
            "ones_rd": np.ones((1, DEPTH), dtype=np.float32),
        })
    return in_maps


def _run(inputs, trace=False):
    from concourse.bass_utils import run_bass_kernel_spmd

    nc = _get_program()
    in_maps = _make_in_maps(**inputs)
    res = run_bass_kernel_spmd(
        nc, in_maps, core_ids=list(range(CORES)), trace=trace,
    )
    outs = [res.results[c]["out"].astype(np.float64) for c in range(CORES)]
    full = np.empty((B, S, D), dtype=np.float32)
    for b in range(B):
        acc = outs[4 * b]
        for hg in range(1, HG):
            acc = acc + outs[4 * b + hg]
        full[b] = acc.astype(np.float32)
    return full, res


def kernel(query, attention_mask, Wq, Wk, Wv, Wo):
    full, _ = _run(dict(
        query=np.asarray(query), attention_mask=np.asarray(attention_mask),
        Wq=np.asarray(Wq), Wk=np.asarray(Wk), Wv=np.asarray(Wv),
        Wo=np.asarray(Wo),
    ))
    return full

